# revision 16
# baseline (speedup 1.0000x reference)
"""Trainium2 Bass kernel for DistanceGatedScoringFunction.

Computation (per row n of the batch):
  gl     = gate_input @ Wg + bg                       [L]
  logits = -(||gl||^2 - 2 gl @ centers.T + ||c||^2)   [E]
  logits = relu(logits @ Wgm1 + bgm1) @ Wgm2 + bgm2   [E]
  probs  = softmax(logits + gumbel)                   [E]
  eo_e   = (relu(relu(x @ We1_e + be1_e) @ We2_e + be2_e)) @ We3_e + be3_e
  out    = sigmoid(sum_e eo_e * probs_e)              [1]

Strategy: data-parallel over 8 NeuronCores (shard N), replicate params.
Feature-major layout (features on partitions, tokens on the free dim).

Expert MLPs run in fp8 e4m3 with DoubleRow matmuls: one instruction
contracts 256 (2x128 chunks) at ~0.42ns/row -- 2x fp32r math throughput.
Host quantization (validated ~9e-3 rel in numpy):
  x/4 -> e4m3, 4*We1 -> e4m3 (product unscaled);
  We2 columns scaled by |We3[k]|*2^{s_k} (pow2) -> e4m3; the We3 reduction
  then uses exact +/-2^{-s_k} fp8 weights, so We3 is never quantized.

The psum->sbuf relu drains are the vector-side cost and Pool cannot read
PSUM on TRN2, so tiles run in token PAIRS: each (expert, layer, chunk)
gets a 2-bank psum pair covering 1000 tokens and ONE drain op with a
uniform per-partition bias; drains split across ACT and DVE.  Matmul
outputs must sit at partition base 0 (walrus rejects offset quadrants),
so the two tiles' tiny epilogues stack on partitions 0-15/16-31 via
32-wide zero-padded stationaries: one eo psum region holds
[ones(t0); eo(t0); ones(t1); eo(t1)], the gating mlp2 psum holds
[z(t0); z(t0); z(t1); z(t1)], and the den/num reduction is a single
[32 -> 4] matmul.  Gating stays fp32r; softmax is unnormalized
(w = exp(z - approx-center)); den/num bounce to DRAM for a batched
division+sigmoid post-pass.
"""

import numpy as np

N, D, H, E, L = 100000, 256, 256, 8, 64
M_CORES = 8
NC_N = N // M_CORES     # rows per core
F = 500                 # token tile (matmul moving free dim)
T_TILES = NC_N // F     # 25
N_GRP = (T_TILES + 1) // 2   # 13 gumbel column-groups
LG_SHIFT = 90.0


def _build_nc(nc_n, f):
    """Build and compile the single-core Bass program (shared by all cores)."""
    from contextlib import ExitStack

    import concourse.bacc as bacc
    import concourse.mybir as mybir
    import concourse.tile as tile

    fp32 = mybir.dt.float32
    fr = mybir.dt.float32r
    f8 = mybir.dt.float8e4
    AF = mybir.ActivationFunctionType
    OP = mybir.AluOpType
    PM = mybir.MatmulPerfMode
    t_tiles = nc_n // f
    assert t_tiles * f == nc_n
    PP = 100
    PJ = nc_n // PP
    assert PP * PJ == nc_n

    nc = bacc.Bacc("TRN2", target_bir_lowering=False, debug=False)

    # ---- DRAM I/O ----
    xs_d = nc.dram_tensor("xs", [128, 2 * nc_n], f8, kind="ExternalInput")
    xg_d = nc.dram_tensor("xg", [128, 2 * nc_n], fr, kind="ExternalInput")
    gm_d = nc.dram_tensor("gm", [32, N_GRP * f], fp32, kind="ExternalInput")
    we1_d = nc.dram_tensor("we1", [128, E * 2 * 2 * 128], f8, kind="ExternalInput")
    we2_d = nc.dram_tensor("we2", [128, E * 2 * 2 * 128], f8, kind="ExternalInput")
    we3_d = nc.dram_tensor("we3", [128, E * 2 * 2 * 32], f8, kind="ExternalInput")
    wg_d = nc.dram_tensor("wg", [128, 2 * 128], fr, kind="ExternalInput")
    wdist_d = nc.dram_tensor("wdist", [128, E], fr, kind="ExternalInput")
    wgm1_d = nc.dram_tensor("wgm1", [E, 2 * 128], fr, kind="ExternalInput")
    wgm2_d = nc.dram_tensor("wgm2", [128, 2 * 2 * 32], fr, kind="ExternalInput")
    dnw_d = nc.dram_tensor("dnw", [32, 4], fr, kind="ExternalInput")
    b128_d = nc.dram_tensor("b128", [128, 35], fp32, kind="ExternalInput")
    csm_d = nc.dram_tensor("csm", [32, 4], fp32, kind="ExternalInput")
    out_d = nc.dram_tensor("out", [nc_n], fp32, kind="ExternalOutput")
    scr_d = nc.dram_tensor("scr", [2, nc_n], fp32)  # den/num bounce

    xs_r = xs_d.ap().rearrange("p (c n) -> p c n", c=2)
    xg_r = xg_d.ap().rearrange("p (c n) -> p c n", c=2)

    with tile.TileContext(nc) as tc, ExitStack() as ctx:
        cw = ctx.enter_context(tc.tile_pool(name="cw", bufs=1))
        xin = ctx.enter_context(tc.tile_pool(name="xin", bufs=3))
        wk = ctx.enter_context(tc.tile_pool(name="wk", bufs=2))
        h8 = ctx.enter_context(tc.tile_pool(name="h8", bufs=4))
        hgp = ctx.enter_context(tc.tile_pool(name="hgp", bufs=2))
        pbig = ctx.enter_context(tc.tile_pool(name="pbig", bufs=3, space="PSUM"))
        pglp = ctx.enter_context(tc.tile_pool(name="pglp", bufs=1, space="PSUM"))
        peo = ctx.enter_context(tc.tile_pool(name="peo", bufs=1, space="PSUM"))

        # ---- constants into SBUF (one DMA each) ----
        we1_s = cw.tile([128, E, 2, 2, 128], f8)
        nc.sync.dma_start(out=we1_s, in_=we1_d.ap())
        we2_s = cw.tile([128, E, 2, 2, 128], f8)
        nc.sync.dma_start(out=we2_s, in_=we2_d.ap())
        we3_s = cw.tile([128, E, 2, 2, 32], f8)
        nc.sync.dma_start(out=we3_s, in_=we3_d.ap())
        wg_s = cw.tile([128, 2, 128], fr)
        nc.sync.dma_start(out=wg_s, in_=wg_d.ap())
        wdist_s = cw.tile([128, E], fr)
        nc.sync.dma_start(out=wdist_s, in_=wdist_d.ap())
        wgm1_s = cw.tile([E, 2, 128], fr)
        nc.sync.dma_start(out=wgm1_s, in_=wgm1_d.ap())
        wgm2_s = cw.tile([128, 2, 2, 32], fr)
        nc.sync.dma_start(out=wgm2_s, in_=wgm2_d.ap())
        dnw_s = cw.tile([32, 4], fr)
        nc.sync.dma_start(out=dnw_s, in_=dnw_d.ap())
        b128_s = cw.tile([128, 35], fp32)
        nc.sync.dma_start(out=b128_s, in_=b128_d.ap())
        csm_s = cw.tile([32, 4], fp32)
        nc.sync.dma_start(out=csm_s, in_=csm_d.ap())

        # drain engine rotation (ACT also carries the gating chain)
        DRAIN_PAT = ["dve", "act", "dve", "act", "dve", "act", "dve", "act",
                     "dve", "act", "dve", "act", "dve", "act", "dve", "dve"]

        def drain(idx, out, in_, bias):
            if DRAIN_PAT[idx % len(DRAIN_PAT)] == "act":
                nc.scalar.activation(out, in_, AF.Relu, bias=bias)
            else:
                nc.vector.tensor_scalar(out=out, in0=in_, scalar1=bias,
                                        scalar2=0.0, op0=OP.add, op1=OP.max)

        # Gating for group i runs interleaved into group i-1's expert loop
        # (software pipelining) so the serial psum chain never stalls the PE.
        def make_stages(gi, n0, g):
            """Return (ctx, [stage callbacks]) for one group's gating chain."""
            ntk = g * f
            gs = {"g": g, "n0": n0}

            def s_dma():
                xs_t = xin.tile([128, 2, ntk], f8, tag="xs", name="xs_t")
                nc.sync.dma_start(out=xs_t, in_=xs_r[:, :, n0 : n0 + ntk])
                xg_t = xin.tile([128, 2, ntk], fr, tag="xg", name="xg_t")
                nc.sync.dma_start(out=xg_t, in_=xg_r[:, :, n0 : n0 + ntk])
                gm_t = xin.tile([32, f], fp32, tag="gm", name="gm_t")
                nc.sync.dma_start(out=gm_t, in_=gm_d.ap()[:, gi * f : (gi + 1) * f])
                gs["xs"], gs["xg"], gs["gm"] = xs_t, xg_t, gm_t

            def g_gate(ti):
                pglg = pglp.tile([128, 512], fp32, tag="pg", name="pglg")
                for c in range(2):
                    nc.tensor.matmul(pglg[:, 0:f], wg_s[:, c, :],
                                     gs["xg"][:, c, ti * f : (ti + 1) * f],
                                     start=(c == 0), stop=(c == 1))
                glgs = wk.tile([128, f], fr, tag=f"glgs{ti}", name="glgs")
                nc.scalar.activation(glgs[0:L, :], pglg[0:L, 0:f], AF.Identity,
                                     bias=b128_s[0:L, 34:35])
                nc.scalar.activation(glgs[L:128, :], pglg[L:128, 0:f], AF.Square,
                                     bias=b128_s[L:128, 34:35])
                gs[f"glgs{ti}"] = glgs

            def g_dist(ti):
                plg = pglp.tile([E, 512], fp32, tag="pg", name="plg")
                nc.tensor.matmul(plg[:, 0:f], wdist_s, gs[f"glgs{ti}"],
                                 start=True, stop=True)
                lg_t = wk.tile([E, f], fr, tag=f"lg{ti}", name="lg_t")
                nc.scalar.activation(lg_t, plg[:, 0:f], AF.Identity,
                                     bias=csm_s[0:E, 1:2])
                gs[f"lg{ti}"] = lg_t

            def g_mlp1():
                # token-paired: psum pair (hc fixed, ti varies), uniform bias
                hgsu = hgp.tile([128, 2, g, f], fr, tag="hg", name="hgsu")
                for hc in range(2):
                    ph = pbig.tile([128, 2, 512], fp32, tag="pb", name="phg")
                    for ti in range(g):
                        nc.tensor.matmul(ph[:, ti, 0:f], wgm1_s[:, hc, :],
                                         gs[f"lg{ti}"], start=True, stop=True)
                    nc.scalar.activation(hgsu[:, hc, :, :], ph[:, 0:g, 0:f],
                                         AF.Relu,
                                         bias=b128_s[:, 32 + hc : 33 + hc])
                gs["hg"] = hgsu

            def g_mlp2():
                # p1c rows: [z(t0); z(t0); z(t1); z(t1)] via 32-wide stationaries
                p1t = pglp.tile([32, 512], fp32, tag="pg", name="p1t")
                p1 = p1t[:, 0:f]
                nmm = 2 * g
                k = 0
                for ti in range(g):
                    for hc in range(2):
                        nc.tensor.matmul(p1, wgm2_s[:, ti, hc, :],
                                         gs["hg"][:, hc, ti, :],
                                         start=(k == 0), stop=(k == nmm - 1))
                        k += 1
                z2 = wk.tile([32, f], fp32, tag="z2", name="z2")
                nc.vector.tensor_add(z2, p1, gs["gm"])
                w2 = wk.tile([32, f], fr, tag="w2", name="w2")
                nc.scalar.activation(w2, z2, AF.Exp, bias=csm_s[:, 2:3])
                gs["w2"] = w2

            if g == 2:
                st_list = [s_dma, lambda: g_gate(0), lambda: g_dist(0),
                           lambda: g_gate(1), lambda: g_dist(1),
                           g_mlp1, g_mlp2]
            else:
                st_list = [s_dma, lambda: g_gate(0), lambda: g_dist(0),
                           g_mlp1, g_mlp2]
            return gs, st_list

        def emit_experts(gs, next_stages, nd0):
            """Expert fp8 branch + epilogue for one group, interleaving the
            NEXT group's gating stage callbacks."""
            g, n0 = gs["g"], gs["n0"]
            xs_t = gs["xs"]
            # eo psum rows: [ones(t0); eo(t0); ones(t1); eo(t1)]
            eo_m = peo.tile([32, 512], fp32, tag="eo", name="eo_m")
            nd = nd0
            n_eo = 0
            for e in range(E):
                h1su = h8.tile([128, 2, g, f], f8, tag="h1", name="h1su")
                for hc in range(2):
                    ph = pbig.tile([128, 2, 512], fp32, tag="pb", name="ph1")
                    for ti in range(g):
                        nc.tensor.matmul(
                            ph[:, ti, 0:f], we1_s[:, e, hc, :, :],
                            xs_t[:, :, ti * f : (ti + 1) * f],
                            start=True, stop=True, perf_mode=PM.DoubleRow)
                    drain(nd, h1su[:, hc, :, :], ph[:, 0:g, 0:f],
                          b128_s[:, e * 2 + hc : e * 2 + hc + 1])
                    nd += 1
                h2su = h8.tile([128, 2, g, f], f8, tag="h2", name="h2su")
                for kc in range(2):
                    ph = pbig.tile([128, 2, 512], fp32, tag="pb", name="ph2")
                    for ti in range(g):
                        nc.tensor.matmul(
                            ph[:, ti, 0:f], we2_s[:, e, kc, :, :],
                            h1su[:, :, ti, :],
                            start=True, stop=True, perf_mode=PM.DoubleRow)
                    drain(nd, h2su[:, kc, :, :], ph[:, 0:g, 0:f],
                          b128_s[:, 16 + e * 2 + kc : 17 + e * 2 + kc])
                    nd += 1
                for ti in range(g):
                    nc.tensor.matmul(eo_m[:, 0:f], we3_s[:, e, ti, :, :],
                                     h2su[:, :, ti, :],
                                     start=(n_eo == 0), stop=(n_eo == g * E - 1),
                                     perf_mode=PM.DoubleRow)
                    n_eo += 1
                if e < len(next_stages):
                    next_stages[e]()

            # ---- [w; eo*w] (tiles stacked) -> den/num -> DRAM bounce ----
            eo2s = wk.tile([32, f], fp32, tag="eo2s", name="eo2s")
            nc.vector.tensor_scalar(out=eo2s, in0=eo_m[:, 0:f],
                                    scalar1=csm_s[:, 3:4], scalar2=None,
                                    op0=OP.add)
            ewp2 = wk.tile([32, f], fr, tag="ewp2", name="ewp2")
            nc.gpsimd.tensor_mul(ewp2, eo2s, gs["w2"].bitcast(fp32))
            # den/num reuse rows 0-3 of the eo bank (a fresh accumulation group)
            nc.tensor.matmul(eo_m[0 : 2 * g, 0:f], dnw_s[:, 0 : 2 * g], ewp2,
                             start=True, stop=True)
            dn_s = wk.tile([2 * g, f], fp32, tag="dns", name="dn_s")
            nc.scalar.activation(dn_s, eo_m[0 : 2 * g, 0:f], AF.Identity)
            for ti in range(g):
                nc.sync.dma_start(
                    out=scr_d.ap()[:, n0 + ti * f : n0 + (ti + 1) * f],
                    in_=dn_s[2 * ti : 2 * ti + 2, :])
            return nd

        groups = [(st, st * 2 * f, 2) for st in range(t_tiles // 2)]
        if t_tiles % 2:
            groups.append((t_tiles // 2, (t_tiles - 1) * f, 1))

        gs_cur, st_cur = make_stages(*groups[0])
        for cb in st_cur:
            cb()  # group 0 gating runs un-pipelined (startup)
        nd = 0
        for idx in range(len(groups)):
            if idx + 1 < len(groups):
                gs_next, st_next = make_stages(*groups[idx + 1])
            else:
                gs_next, st_next = None, []
            nd = emit_experts(gs_cur, st_next, nd)
            gs_cur = gs_next

        # ---- post-pass: out = 1 / (1 + exp(-num/den)), full-width ----
        dn2 = cw.tile([PP, 2, PJ], fp32)
        nc.sync.dma_start(out=dn2, in_=scr_d.ap().rearrange("c (p j) -> p c j", p=PP))
        denr2 = cw.tile([PP, PJ], fp32)
        nc.vector.reciprocal(denr2, dn2[:, 0, :])
        rat2 = cw.tile([PP, PJ], fp32)
        nc.vector.tensor_mul(rat2, dn2[:, 1, :], denr2)
        en2 = cw.tile([PP, PJ], fp32)
        nc.scalar.activation(en2, rat2, AF.Exp, scale=-1.0)
        ep2 = cw.tile([PP, PJ], fp32)
        nc.vector.tensor_scalar_add(ep2, en2, 1.0)
        outp = cw.tile([PP, PJ], fp32)
        nc.vector.reciprocal(outp, ep2)
        nc.sync.dma_start(out=out_d.ap().rearrange("(p j) -> p j", p=PP), in_=outp)

    nc.compile()
    return nc


def _pack_weights(ins):
    """Host-side packing of parameters into SBUF-ready layouts."""
    import ml_dtypes

    f32 = np.float32
    f8np = ml_dtypes.float8_e4m3
    We1, be1 = np.asarray(ins["We1"], f32), np.asarray(ins["be1"], f32)
    We2, be2 = np.asarray(ins["We2"], f32), np.asarray(ins["be2"], f32)
    We3, be3 = np.asarray(ins["We3"], f32), np.asarray(ins["be3"], f32)
    Wg, bg = np.asarray(ins["Wg"], f32), np.asarray(ins["bg"], f32)
    centers = np.asarray(ins["centers"], f32)
    Wgm1, bgm1 = np.asarray(ins["Wgm1"], f32), np.asarray(ins["bgm1"], f32)
    Wgm2, bgm2 = np.asarray(ins["Wgm2"], f32), np.asarray(ins["bgm2"], f32)

    # --- expert weights, fp8 scheme ---
    we1_p = (4.0 * We1).reshape(E, 2, 128, 2, 128)       # [e, c, p, hc, j]
    we1_p = we1_p.transpose(2, 0, 3, 1, 4).reshape(128, E * 2 * 2 * 128)
    we1_p8 = np.ascontiguousarray(we1_p).astype(f8np)

    we2_p = np.zeros((128, E, 2, 2, 128), f32)
    be2_p = np.zeros((128, E * 2), f32)
    we3_p = np.zeros((128, E, 2, 2, 32), f32)            # [p, e, ti, c, col]
    for e in range(E):
        w3 = We3[e]                                       # [H]
        col = We2[e] * np.abs(w3)[None, :]                # [H, H]
        colmax = np.abs(col).max(axis=0)
        s = np.clip(np.floor(np.log2(0.1875 / np.maximum(colmax, 1e-30))), 0, 9)
        sc = 2.0 ** s
        colq = col * sc[None, :]
        be2e = be2[e] * np.abs(w3) * sc
        sgn = np.sign(w3) * (2.0 ** (-s))
        for kc in range(2):
            ksl = slice(kc * 128, (kc + 1) * 128)
            blk = colq[:, ksl].reshape(2, 128, 128)       # [c, p, j]
            we2_p[:, e, kc, :, :] = blk.transpose(1, 0, 2)
            be2_p[:, e * 2 + kc] = be2e[ksl]
        for ti in range(2):
            for c in range(2):
                we3_p[:, e, ti, c, 16 * ti + 8 + e] = sgn[c * 128 : (c + 1) * 128]
    we2_p8 = np.ascontiguousarray(we2_p.reshape(128, -1)).astype(f8np)
    we3_p8 = np.ascontiguousarray(we3_p.reshape(128, -1)).astype(f8np)
    assert np.all(we3_p8.astype(f32) == we3_p.reshape(128, -1)), "we3 pow2 inexact"

    # --- gating weights (fp32) ---
    wg_p = np.zeros((128, 2, 128), f32)
    wgr = Wg.reshape(2, 128, L).transpose(1, 0, 2)        # [p, c, l]
    wg_p[:, :, 0:L] = wgr
    wg_p[:, :, L:128] = wgr
    wdist = np.zeros((128, E), f32)
    wdist[0:L, :] = 2.0 * centers.T
    wdist[L:128, :] = -1.0
    wgm1_p = np.ascontiguousarray(Wgm1.reshape(E, 2 * 128))
    W2c = Wgm2 - Wgm2.mean(axis=1, keepdims=True)
    w2r = W2c.reshape(2, 128, E).transpose(1, 0, 2)       # [p, hc, e]
    wgm2_p = np.zeros((128, 2, 2, 32), f32)               # [p, ti, hc, col]
    for ti in range(2):
        for hc in range(2):
            wgm2_p[:, ti, hc, 16 * ti + 0 : 16 * ti + 8] = w2r[:, hc, :]
            wgm2_p[:, ti, hc, 16 * ti + 8 : 16 * ti + 16] = w2r[:, hc, :]
    dnw = np.zeros((32, 4), f32)
    for ti in range(2):
        dnw[16 * ti : 16 * ti + 8, 2 * ti + 0] = 1.0      # den(ti)
        dnw[16 * ti + 8 : 16 * ti + 16, 2 * ti + 1] = 1.0  # num(ti)

    b128 = np.zeros((128, 35), f32)
    b128[:, 0:16] = be1.reshape(E, 2, 128).transpose(2, 0, 1).reshape(128, 16)
    b128[:, 16:32] = be2_p
    b128[:, 32:34] = (bgm1 - LG_SHIFT * Wgm1.sum(axis=0)).reshape(2, 128).T
    b128[:, 34] = np.concatenate([bg, bg])
    csm = np.zeros((32, 4), f32)
    csm[0:E, 1] = -(centers * centers).sum(axis=1) + LG_SHIFT
    for ti in range(2):
        csm[16 * ti : 16 * ti + 16, 2] = np.concatenate([bgm2, bgm2])
        csm[16 * ti : 16 * ti + 8, 3] = 1.0
        csm[16 * ti + 8 : 16 * ti + 16, 3] = be3
    return {
        "we1": we1_p8, "we2": we2_p8, "we3": we3_p8,
        "wg": np.ascontiguousarray(wg_p.reshape(128, -1)),
        "wdist": wdist, "wgm1": wgm1_p,
        "wgm2": np.ascontiguousarray(wgm2_p.reshape(128, -1)),
        "dnw": dnw, "b128": b128, "csm": csm,
    }


def make_in_maps(inputs):
    """Full host-side prep: weights + per-core sharded activations."""
    import ml_dtypes

    f32 = np.float32
    f8np = ml_dtypes.float8_e4m3
    wmaps = _pack_weights(inputs)
    x = np.asarray(inputs["score_input"], f32)
    xg = np.asarray(inputs["gate_input"], f32)
    gm = np.asarray(inputs["gumbel_noise"], f32)

    # [N, D] -> [p, c, n] with d = c*128 + p
    xs8 = np.ascontiguousarray(
        (x.T / 4.0).reshape(2, 128, N).transpose(1, 0, 2)).astype(f8np)
    xgt = np.ascontiguousarray(xg.T.reshape(2, 128, N).transpose(1, 0, 2))
    gmT = gm.T                                            # [E, N]

    in_maps = []
    for c in range(M_CORES):
        s = slice(c * NC_N, (c + 1) * NC_N)
        m = dict(wmaps)
        m["xs"] = np.ascontiguousarray(xs8[:, :, s].reshape(128, -1))
        m["xg"] = np.ascontiguousarray(xgt[:, :, s].reshape(128, -1))
        # gm32: rows 0-15 = [gm;gm] of even tile, rows 16-31 = odd tile
        gmc = gmT[:, s]                                   # [E, NC_N]
        gm32 = np.zeros((32, N_GRP * F), f32)
        for st in range(T_TILES // 2):
            e0 = gmc[:, (2 * st) * F : (2 * st + 1) * F]
            o0 = gmc[:, (2 * st + 1) * F : (2 * st + 2) * F]
            gm32[0:8, st * F : (st + 1) * F] = e0
            gm32[8:16, st * F : (st + 1) * F] = e0
            gm32[16:24, st * F : (st + 1) * F] = o0
            gm32[24:32, st * F : (st + 1) * F] = o0
        if T_TILES % 2:
            lt = gmc[:, (T_TILES - 1) * F :]
            gm32[0:8, (N_GRP - 1) * F :] = lt
            gm32[8:16, (N_GRP - 1) * F :] = lt
        m["gm"] = gm32
        in_maps.append(m)
    return in_maps


_NC_CACHE = {}


def _get_nc(nc_n, f):
    key = (nc_n, f)
    if key not in _NC_CACHE:
        _NC_CACHE[key] = _build_nc(nc_n, f)
    return _NC_CACHE[key]


def kernel(**inputs) -> np.ndarray:
    from concourse.bass_utils import run_bass_kernel_spmd

    nc = _get_nc(NC_N, F)
    in_maps = make_in_maps(inputs)
    res = run_bass_kernel_spmd(nc, in_maps, core_ids=list(range(M_CORES)))
    out = np.concatenate([res.results[c]["out"] for c in range(M_CORES)])
    return out.reshape(N, 1).astype(np.float32)


if __name__ == "__main__":
    import jax

    with jax.default_device(jax.local_devices(backend="cpu")[0]):
        import reference

        ins = reference.setup_inputs()
        ins = {k: np.asarray(v) for k, v in ins.items()}
        expected = np.asarray(reference.reference(**ins))
    out = kernel(**ins)
    err = np.abs(out - expected).max()
    print("max abs err:", err, "rel:", err / np.abs(expected).max())


# revision 17
# speedup vs baseline: 1.1861x; 1.1861x over previous
"""Trainium2 Bass kernel for DistanceGatedScoringFunction.

Computation (per row n of the batch):
  gl     = gate_input @ Wg + bg                       [L]
  logits = -(||gl||^2 - 2 gl @ centers.T + ||c||^2)   [E]
  logits = relu(logits @ Wgm1 + bgm1) @ Wgm2 + bgm2   [E]
  probs  = softmax(logits + gumbel)                   [E]
  eo_e   = (relu(relu(x @ We1_e + be1_e) @ We2_e + be2_e)) @ We3_e + be3_e
  out    = sigmoid(sum_e eo_e * probs_e)              [1]

Strategy: data-parallel over 8 NeuronCores (shard N), replicate params.
Feature-major layout (features on partitions, tokens on the free dim).

Expert MLPs run in fp8 e4m3 with DoubleRow matmuls: one instruction
contracts 256 (2x128 chunks) at ~0.42ns/row -- 2x fp32r math throughput.
Host quantization (validated ~9e-3 rel in numpy):
  x/4 -> e4m3, 4*We1 -> e4m3 (product unscaled);
  We2 columns scaled by |We3[k]|*2^{s_k} (pow2) -> e4m3; the We3 reduction
  then uses exact +/-2^{-s_k} fp8 weights, so We3 is never quantized.

The psum->sbuf relu drains are the vector-side cost and Pool cannot read
PSUM on TRN2, so tiles run in token PAIRS: each (expert, layer, chunk)
gets a 2-bank psum pair covering 1000 tokens and ONE drain op with a
uniform per-partition bias; drains split across ACT and DVE.  Matmul
outputs must sit at partition base 0 (walrus rejects offset quadrants),
so the two tiles' tiny epilogues stack on partitions 0-15/16-31 via
32-wide zero-padded stationaries: one eo psum region holds
[ones(t0); eo(t0); ones(t1); eo(t1)], the gating mlp2 psum holds
[z(t0); z(t0); z(t1); z(t1)], and the den/num reduction is a single
[32 -> 4] matmul.  Gating stays fp32r; softmax is unnormalized
(w = exp(z - approx-center)); den/num bounce to DRAM for a batched
division+sigmoid post-pass.
"""

import numpy as np

N, D, H, E, L = 100000, 256, 256, 8, 64
M_CORES = 8
NC_N = N // M_CORES     # rows per core
F = 500                 # token tile (matmul moving free dim)
T_TILES = NC_N // F     # 25
N_GRP = (T_TILES + 1) // 2   # 13 gumbel column-groups
LG_SHIFT = 90.0


def _build_nc(nc_n, f):
    """Build and compile the single-core Bass program (shared by all cores)."""
    from contextlib import ExitStack

    import concourse.bacc as bacc
    import concourse.mybir as mybir
    import concourse.tile as tile

    fp32 = mybir.dt.float32
    fr = mybir.dt.float32r
    f8 = mybir.dt.float8e4
    AF = mybir.ActivationFunctionType
    OP = mybir.AluOpType
    PM = mybir.MatmulPerfMode
    t_tiles = nc_n // f
    assert t_tiles * f == nc_n
    PP = 100
    PJ = nc_n // PP
    assert PP * PJ == nc_n

    nc = bacc.Bacc("TRN2", target_bir_lowering=False, debug=False)

    # ---- DRAM I/O ----
    xs_d = nc.dram_tensor("xs", [128, 2 * nc_n], f8, kind="ExternalInput")
    xg_d = nc.dram_tensor("xg", [128, 2 * nc_n], fr, kind="ExternalInput")
    gm_d = nc.dram_tensor("gm", [32, N_GRP * f], fp32, kind="ExternalInput")
    we1_d = nc.dram_tensor("we1", [128, E * 2 * 2 * 128], f8, kind="ExternalInput")
    we2_d = nc.dram_tensor("we2", [128, E * 2 * 2 * 128], f8, kind="ExternalInput")
    we3_d = nc.dram_tensor("we3", [128, E * 2 * 2 * 32], f8, kind="ExternalInput")
    wg_d = nc.dram_tensor("wg", [128, 2 * 128], fr, kind="ExternalInput")
    wdist_d = nc.dram_tensor("wdist", [128, E], fr, kind="ExternalInput")
    wgm1_d = nc.dram_tensor("wgm1", [E, 2 * 128], fr, kind="ExternalInput")
    wgm2_d = nc.dram_tensor("wgm2", [128, 2 * 2 * 32], fr, kind="ExternalInput")
    dnw_d = nc.dram_tensor("dnw", [32, 4], fr, kind="ExternalInput")
    b128_d = nc.dram_tensor("b128", [128, 35], fp32, kind="ExternalInput")
    csm_d = nc.dram_tensor("csm", [32, 4], fp32, kind="ExternalInput")
    out_d = nc.dram_tensor("out", [nc_n], fp32, kind="ExternalOutput")
    scr_d = nc.dram_tensor("scr", [2, nc_n], fp32)  # den/num bounce

    xs_r = xs_d.ap().rearrange("p (c n) -> p c n", c=2)
    xg_r = xg_d.ap().rearrange("p (c n) -> p c n", c=2)

    with tile.TileContext(nc) as tc, ExitStack() as ctx:
        cw = ctx.enter_context(tc.tile_pool(name="cw", bufs=1))
        xin = ctx.enter_context(tc.tile_pool(name="xin", bufs=3))
        wk = ctx.enter_context(tc.tile_pool(name="wk", bufs=2))
        h8 = ctx.enter_context(tc.tile_pool(name="h8", bufs=4))
        hgp = ctx.enter_context(tc.tile_pool(name="hgp", bufs=2))
        pbig = ctx.enter_context(tc.tile_pool(name="pbig", bufs=3, space="PSUM"))
        pglp = ctx.enter_context(tc.tile_pool(name="pglp", bufs=1, space="PSUM"))
        peo = ctx.enter_context(tc.tile_pool(name="peo", bufs=1, space="PSUM"))

        # ---- constants into SBUF (one DMA each) ----
        we1_s = cw.tile([128, E, 2, 2, 128], f8)
        nc.sync.dma_start(out=we1_s, in_=we1_d.ap())
        we2_s = cw.tile([128, E, 2, 2, 128], f8)
        nc.sync.dma_start(out=we2_s, in_=we2_d.ap())
        we3_s = cw.tile([128, E, 2, 2, 32], f8)
        nc.sync.dma_start(out=we3_s, in_=we3_d.ap())
        wg_s = cw.tile([128, 2, 128], fr)
        nc.sync.dma_start(out=wg_s, in_=wg_d.ap())
        wdist_s = cw.tile([128, E], fr)
        nc.sync.dma_start(out=wdist_s, in_=wdist_d.ap())
        wgm1_s = cw.tile([E, 2, 128], fr)
        nc.sync.dma_start(out=wgm1_s, in_=wgm1_d.ap())
        wgm2_s = cw.tile([128, 2, 2, 32], fr)
        nc.sync.dma_start(out=wgm2_s, in_=wgm2_d.ap())
        dnw_s = cw.tile([32, 4], fr)
        nc.sync.dma_start(out=dnw_s, in_=dnw_d.ap())
        b128_s = cw.tile([128, 35], fp32)
        nc.sync.dma_start(out=b128_s, in_=b128_d.ap())
        csm_s = cw.tile([32, 4], fp32)
        nc.sync.dma_start(out=csm_s, in_=csm_d.ap())

        # drain engine rotation (ACT also carries the gating chain)
        DRAIN_PAT = ["dve", "act", "dve", "act", "dve", "act", "dve", "act",
                     "dve", "act", "dve", "act", "dve", "act", "dve", "dve"]

        def drain(idx, out, in_, bias):
            if DRAIN_PAT[idx % len(DRAIN_PAT)] == "act":
                nc.scalar.activation(out, in_, AF.Relu, bias=bias)
            else:
                nc.vector.tensor_scalar(out=out, in0=in_, scalar1=bias,
                                        scalar2=0.0, op0=OP.add, op1=OP.max)

        # Gating for group i runs interleaved into group i-1's expert loop
        # (software pipelining) so the serial psum chain never stalls the PE.
        def make_stages(gi, n0, g):
            """Return (ctx, [stage callbacks]) for one group's gating chain."""
            ntk = g * f
            gs = {"g": g, "n0": n0}

            def s_dma():
                xs_t = xin.tile([128, 2, ntk], f8, tag="xs", name="xs_t")
                nc.sync.dma_start(out=xs_t, in_=xs_r[:, :, n0 : n0 + ntk])
                xg_t = xin.tile([128, 2, ntk], fr, tag="xg", name="xg_t")
                nc.sync.dma_start(out=xg_t, in_=xg_r[:, :, n0 : n0 + ntk])
                gm_t = xin.tile([32, f], fp32, tag="gm", name="gm_t")
                nc.sync.dma_start(out=gm_t, in_=gm_d.ap()[:, gi * f : (gi + 1) * f])
                gs["xs"], gs["xg"], gs["gm"] = xs_t, xg_t, gm_t

            def g_gate(ti):
                pglg = pglp.tile([128, 512], fp32, tag="pg", name="pglg")
                for c in range(2):
                    nc.tensor.matmul(pglg[:, 0:f], wg_s[:, c, :],
                                     gs["xg"][:, c, ti * f : (ti + 1) * f],
                                     start=(c == 0), stop=(c == 1))
                glgs = wk.tile([128, f], fr, tag=f"glgs{ti}", name="glgs")
                nc.scalar.activation(glgs[0:L, :], pglg[0:L, 0:f], AF.Identity,
                                     bias=b128_s[0:L, 34:35])
                nc.scalar.activation(glgs[L:128, :], pglg[L:128, 0:f], AF.Square,
                                     bias=b128_s[L:128, 34:35])
                gs[f"glgs{ti}"] = glgs

            def g_dist(ti):
                plg = pglp.tile([E, 512], fp32, tag="pg", name="plg")
                nc.tensor.matmul(plg[:, 0:f], wdist_s, gs[f"glgs{ti}"],
                                 start=True, stop=True)
                lg_t = wk.tile([E, f], fr, tag=f"lg{ti}", name="lg_t")
                nc.scalar.activation(lg_t, plg[:, 0:f], AF.Identity,
                                     bias=csm_s[0:E, 1:2])
                gs[f"lg{ti}"] = lg_t

            def g_mlp1():
                # token-paired: psum pair (hc fixed, ti varies), uniform bias
                hgsu = hgp.tile([128, 2, g, f], fr, tag="hg", name="hgsu")
                for hc in range(2):
                    ph = pbig.tile([128, 2, 512], fp32, tag="pb", name="phg")
                    for ti in range(g):
                        nc.tensor.matmul(ph[:, ti, 0:f], wgm1_s[:, hc, :],
                                         gs[f"lg{ti}"], start=True, stop=True)
                    nc.scalar.activation(hgsu[:, hc, :, :], ph[:, 0:g, 0:f],
                                         AF.Relu,
                                         bias=b128_s[:, 32 + hc : 33 + hc])
                gs["hg"] = hgsu

            def g_mlp2():
                # p1c rows: [z(t0); z(t0); z(t1); z(t1)] via 32-wide stationaries
                p1t = pglp.tile([32, 512], fp32, tag="pg", name="p1t")
                p1 = p1t[:, 0:f]
                nmm = 2 * g
                k = 0
                for ti in range(g):
                    for hc in range(2):
                        nc.tensor.matmul(p1, wgm2_s[:, ti, hc, :],
                                         gs["hg"][:, hc, ti, :],
                                         start=(k == 0), stop=(k == nmm - 1))
                        k += 1
                z2 = wk.tile([32, f], fp32, tag="z2", name="z2")
                nc.vector.tensor_add(z2, p1, gs["gm"])
                w2 = wk.tile([32, f], fr, tag="w2", name="w2")
                nc.scalar.activation(w2, z2, AF.Exp, bias=csm_s[:, 2:3])
                gs["w2"] = w2

            if g == 2:
                st_list = [s_dma, lambda: g_gate(0), lambda: g_dist(0),
                           lambda: g_gate(1), lambda: g_dist(1),
                           g_mlp1, g_mlp2]
            else:
                st_list = [s_dma, lambda: g_gate(0), lambda: g_dist(0),
                           g_mlp1, g_mlp2]
            return gs, st_list

        def emit_experts(gs, next_stages, nd0):
            """Expert fp8 branch + epilogue for one group, interleaving the
            NEXT group's gating stage callbacks."""
            g, n0 = gs["g"], gs["n0"]
            xs_t = gs["xs"]
            # eo psum rows: [ones(t0); eo(t0); ones(t1); eo(t1)]
            eo_m = peo.tile([32, 512], fp32, tag="eo", name="eo_m")
            nd = nd0
            n_eo = 0
            for e in range(E):
                h1su = h8.tile([128, 2, g, f], f8, tag="h1", name="h1su")
                for hc in range(2):
                    ph = pbig.tile([128, 2, 512], fp32, tag="pb", name="ph1")
                    for ti in range(g):
                        nc.tensor.matmul(
                            ph[:, ti, 0:f], we1_s[:, e, hc, :, :],
                            xs_t[:, :, ti * f : (ti + 1) * f],
                            start=True, stop=True, perf_mode=PM.DoubleRow)
                    drain(nd, h1su[:, hc, :, :], ph[:, 0:g, 0:f],
                          b128_s[:, e * 2 + hc : e * 2 + hc + 1])
                    nd += 1
                h2su = h8.tile([128, 2, g, f], f8, tag="h2", name="h2su")
                for kc in range(2):
                    ph = pbig.tile([128, 2, 512], fp32, tag="pb", name="ph2")
                    for ti in range(g):
                        nc.tensor.matmul(
                            ph[:, ti, 0:f], we2_s[:, e, kc, :, :],
                            h1su[:, :, ti, :],
                            start=True, stop=True, perf_mode=PM.DoubleRow)
                    drain(nd, h2su[:, kc, :, :], ph[:, 0:g, 0:f],
                          b128_s[:, 16 + e * 2 + kc : 17 + e * 2 + kc])
                    nd += 1
                for ti in range(g):
                    nc.tensor.matmul(eo_m[:, 0:f], we3_s[:, e, ti, :, :],
                                     h2su[:, :, ti, :],
                                     start=(n_eo == 0), stop=(n_eo == g * E - 1),
                                     perf_mode=PM.DoubleRow)
                    n_eo += 1
                if e < len(next_stages):
                    next_stages[e]()

            # ---- [w; eo*w] (tiles stacked) -> den/num -> DRAM bounce ----
            eo2s = wk.tile([32, f], fp32, tag="eo2s", name="eo2s")
            nc.vector.tensor_scalar(out=eo2s, in0=eo_m[:, 0:f],
                                    scalar1=csm_s[:, 3:4], scalar2=None,
                                    op0=OP.add)
            ewp2 = wk.tile([32, f], fr, tag="ewp2", name="ewp2")
            nc.gpsimd.tensor_mul(ewp2, eo2s, gs["w2"].bitcast(fp32))
            pdn = pglp.tile([2 * g, 512], fp32, tag="pg", name="pdn")
            nc.tensor.matmul(pdn[:, 0:f], dnw_s[:, 0 : 2 * g], ewp2,
                             start=True, stop=True)
            dn_s = wk.tile([2 * g, f], fp32, tag="dns", name="dn_s")
            nc.scalar.activation(dn_s, pdn[:, 0:f], AF.Identity)
            for ti in range(g):
                nc.sync.dma_start(
                    out=scr_d.ap()[:, n0 + ti * f : n0 + (ti + 1) * f],
                    in_=dn_s[2 * ti : 2 * ti + 2, :])
            return nd

        groups = [(st, st * 2 * f, 2) for st in range(t_tiles // 2)]
        if t_tiles % 2:
            groups.append((t_tiles // 2, (t_tiles - 1) * f, 1))

        gs_cur, st_cur = make_stages(*groups[0])
        for cb in st_cur:
            cb()  # group 0 gating runs un-pipelined (startup)
        nd = 0
        for idx in range(len(groups)):
            if idx + 1 < len(groups):
                gs_next, st_next = make_stages(*groups[idx + 1])
            else:
                gs_next, st_next = None, []
            nd = emit_experts(gs_cur, st_next, nd)
            gs_cur = gs_next

        # ---- post-pass: out = 1 / (1 + exp(-num/den)), full-width ----
        dn2 = cw.tile([PP, 2, PJ], fp32)
        nc.sync.dma_start(out=dn2, in_=scr_d.ap().rearrange("c (p j) -> p c j", p=PP))
        denr2 = cw.tile([PP, PJ], fp32)
        nc.vector.reciprocal(denr2, dn2[:, 0, :])
        rat2 = cw.tile([PP, PJ], fp32)
        nc.vector.tensor_mul(rat2, dn2[:, 1, :], denr2)
        en2 = cw.tile([PP, PJ], fp32)
        nc.scalar.activation(en2, rat2, AF.Exp, scale=-1.0)
        ep2 = cw.tile([PP, PJ], fp32)
        nc.vector.tensor_scalar_add(ep2, en2, 1.0)
        outp = cw.tile([PP, PJ], fp32)
        nc.vector.reciprocal(outp, ep2)
        nc.sync.dma_start(out=out_d.ap().rearrange("(p j) -> p j", p=PP), in_=outp)

    nc.compile()
    return nc


def _pack_weights(ins):
    """Host-side packing of parameters into SBUF-ready layouts."""
    import ml_dtypes

    f32 = np.float32
    f8np = ml_dtypes.float8_e4m3
    We1, be1 = np.asarray(ins["We1"], f32), np.asarray(ins["be1"], f32)
    We2, be2 = np.asarray(ins["We2"], f32), np.asarray(ins["be2"], f32)
    We3, be3 = np.asarray(ins["We3"], f32), np.asarray(ins["be3"], f32)
    Wg, bg = np.asarray(ins["Wg"], f32), np.asarray(ins["bg"], f32)
    centers = np.asarray(ins["centers"], f32)
    Wgm1, bgm1 = np.asarray(ins["Wgm1"], f32), np.asarray(ins["bgm1"], f32)
    Wgm2, bgm2 = np.asarray(ins["Wgm2"], f32), np.asarray(ins["bgm2"], f32)

    # --- expert weights, fp8 scheme ---
    we1_p = (4.0 * We1).reshape(E, 2, 128, 2, 128)       # [e, c, p, hc, j]
    we1_p = we1_p.transpose(2, 0, 3, 1, 4).reshape(128, E * 2 * 2 * 128)
    we1_p8 = np.ascontiguousarray(we1_p).astype(f8np)

    we2_p = np.zeros((128, E, 2, 2, 128), f32)
    be2_p = np.zeros((128, E * 2), f32)
    we3_p = np.zeros((128, E, 2, 2, 32), f32)            # [p, e, ti, c, col]
    for e in range(E):
        w3 = We3[e]                                       # [H]
        col = We2[e] * np.abs(w3)[None, :]                # [H, H]
        colmax = np.abs(col).max(axis=0)
        s = np.clip(np.floor(np.log2(0.1875 / np.maximum(colmax, 1e-30))), 0, 9)
        sc = 2.0 ** s
        colq = col * sc[None, :]
        be2e = be2[e] * np.abs(w3) * sc
        sgn = np.sign(w3) * (2.0 ** (-s))
        for kc in range(2):
            ksl = slice(kc * 128, (kc + 1) * 128)
            blk = colq[:, ksl].reshape(2, 128, 128)       # [c, p, j]
            we2_p[:, e, kc, :, :] = blk.transpose(1, 0, 2)
            be2_p[:, e * 2 + kc] = be2e[ksl]
        for ti in range(2):
            for c in range(2):
                we3_p[:, e, ti, c, 16 * ti + 8 + e] = sgn[c * 128 : (c + 1) * 128]
    we2_p8 = np.ascontiguousarray(we2_p.reshape(128, -1)).astype(f8np)
    we3_p8 = np.ascontiguousarray(we3_p.reshape(128, -1)).astype(f8np)
    assert np.all(we3_p8.astype(f32) == we3_p.reshape(128, -1)), "we3 pow2 inexact"

    # --- gating weights (fp32) ---
    wg_p = np.zeros((128, 2, 128), f32)
    wgr = Wg.reshape(2, 128, L).transpose(1, 0, 2)        # [p, c, l]
    wg_p[:, :, 0:L] = wgr
    wg_p[:, :, L:128] = wgr
    wdist = np.zeros((128, E), f32)
    wdist[0:L, :] = 2.0 * centers.T
    wdist[L:128, :] = -1.0
    wgm1_p = np.ascontiguousarray(Wgm1.reshape(E, 2 * 128))
    W2c = Wgm2 - Wgm2.mean(axis=1, keepdims=True)
    w2r = W2c.reshape(2, 128, E).transpose(1, 0, 2)       # [p, hc, e]
    wgm2_p = np.zeros((128, 2, 2, 32), f32)               # [p, ti, hc, col]
    for ti in range(2):
        for hc in range(2):
            wgm2_p[:, ti, hc, 16 * ti + 0 : 16 * ti + 8] = w2r[:, hc, :]
            wgm2_p[:, ti, hc, 16 * ti + 8 : 16 * ti + 16] = w2r[:, hc, :]
    dnw = np.zeros((32, 4), f32)
    for ti in range(2):
        dnw[16 * ti : 16 * ti + 8, 2 * ti + 0] = 1.0      # den(ti)
        dnw[16 * ti + 8 : 16 * ti + 16, 2 * ti + 1] = 1.0  # num(ti)

    b128 = np.zeros((128, 35), f32)
    b128[:, 0:16] = be1.reshape(E, 2, 128).transpose(2, 0, 1).reshape(128, 16)
    b128[:, 16:32] = be2_p
    b128[:, 32:34] = (bgm1 - LG_SHIFT * Wgm1.sum(axis=0)).reshape(2, 128).T
    b128[:, 34] = np.concatenate([bg, bg])
    csm = np.zeros((32, 4), f32)
    csm[0:E, 1] = -(centers * centers).sum(axis=1) + LG_SHIFT
    for ti in range(2):
        csm[16 * ti : 16 * ti + 16, 2] = np.concatenate([bgm2, bgm2])
        csm[16 * ti : 16 * ti + 8, 3] = 1.0
        csm[16 * ti + 8 : 16 * ti + 16, 3] = be3
    return {
        "we1": we1_p8, "we2": we2_p8, "we3": we3_p8,
        "wg": np.ascontiguousarray(wg_p.reshape(128, -1)),
        "wdist": wdist, "wgm1": wgm1_p,
        "wgm2": np.ascontiguousarray(wgm2_p.reshape(128, -1)),
        "dnw": dnw, "b128": b128, "csm": csm,
    }


def make_in_maps(inputs):
    """Full host-side prep: weights + per-core sharded activations."""
    import ml_dtypes

    f32 = np.float32
    f8np = ml_dtypes.float8_e4m3
    wmaps = _pack_weights(inputs)
    x = np.asarray(inputs["score_input"], f32)
    xg = np.asarray(inputs["gate_input"], f32)
    gm = np.asarray(inputs["gumbel_noise"], f32)

    # [N, D] -> [p, c, n] with d = c*128 + p
    xs8 = np.ascontiguousarray(
        (x.T / 4.0).reshape(2, 128, N).transpose(1, 0, 2)).astype(f8np)
    xgt = np.ascontiguousarray(xg.T.reshape(2, 128, N).transpose(1, 0, 2))
    gmT = gm.T                                            # [E, N]

    in_maps = []
    for c in range(M_CORES):
        s = slice(c * NC_N, (c + 1) * NC_N)
        m = dict(wmaps)
        m["xs"] = np.ascontiguousarray(xs8[:, :, s].reshape(128, -1))
        m["xg"] = np.ascontiguousarray(xgt[:, :, s].reshape(128, -1))
        # gm32: rows 0-15 = [gm;gm] of even tile, rows 16-31 = odd tile
        gmc = gmT[:, s]                                   # [E, NC_N]
        gm32 = np.zeros((32, N_GRP * F), f32)
        for st in range(T_TILES // 2):
            e0 = gmc[:, (2 * st) * F : (2 * st + 1) * F]
            o0 = gmc[:, (2 * st + 1) * F : (2 * st + 2) * F]
            gm32[0:8, st * F : (st + 1) * F] = e0
            gm32[8:16, st * F : (st + 1) * F] = e0
            gm32[16:24, st * F : (st + 1) * F] = o0
            gm32[24:32, st * F : (st + 1) * F] = o0
        if T_TILES % 2:
            lt = gmc[:, (T_TILES - 1) * F :]
            gm32[0:8, (N_GRP - 1) * F :] = lt
            gm32[8:16, (N_GRP - 1) * F :] = lt
        m["gm"] = gm32
        in_maps.append(m)
    return in_maps


_NC_CACHE = {}


def _get_nc(nc_n, f):
    key = (nc_n, f)
    if key not in _NC_CACHE:
        _NC_CACHE[key] = _build_nc(nc_n, f)
    return _NC_CACHE[key]


def kernel(**inputs) -> np.ndarray:
    from concourse.bass_utils import run_bass_kernel_spmd

    nc = _get_nc(NC_N, F)
    in_maps = make_in_maps(inputs)
    res = run_bass_kernel_spmd(nc, in_maps, core_ids=list(range(M_CORES)))
    out = np.concatenate([res.results[c]["out"] for c in range(M_CORES)])
    return out.reshape(N, 1).astype(np.float32)


if __name__ == "__main__":
    import jax

    with jax.default_device(jax.local_devices(backend="cpu")[0]):
        import reference

        ins = reference.setup_inputs()
        ins = {k: np.asarray(v) for k, v in ins.items()}
        expected = np.asarray(reference.reference(**ins))
    out = kernel(**ins)
    err = np.abs(out - expected).max()
    print("max abs err:", err, "rel:", err / np.abs(expected).max())
